# revision 11
# baseline (speedup 1.0000x reference)
# Triplane FCDecoder kernel for 8x TRN2 NeuronCores.
#
# Math: out[b,n] = sum_{pl in xz,xy,yz} bilinear(plane_pl[b], uv_pl(p[b,n])) . fc_w[:128]
#                  + p[b,n,:] . fc_w[128:131] + fc_b
# Because the decoder is linear, each plane is first projected through
# fc_w[:128] (a [1,128]x[128,HW] matmul), turning 100 MB of plane features
# into twelve 128x128 scalar tables; bilinear sampling then gathers 2x2
# corners of those tables per query point.
#
# Sharding: query points are split 8 ways (12500/batch/core). The projection
# reads each core's 1/8 column shard of all 12 (plane,batch) tables; an
# AllGather replicates the projected tables; each core then samples only its
# own points.

import numpy as np

B, N, C, RES = 4, 100000, 128, 128
NCORES = 8
NP = N // NCORES            # points per batch per core (12500)
M = 392                     # slots per partition (32*392 = 12544 >= NP padded)
NPAD = 32 * M               # padded points per batch per core
J = 16 * M                  # gather indices per Q7 core per instruction
COLS = (RES * RES) // NCORES  # table column shard per core (2048)
PAD = 0.1
EPS = 1e-5

# x = clip(p/(1+PAD+EPS) + 0.5, 0, 1-EPS) * (RES-1), fused into one affine+clip.
_C1 = float(np.float32(RES - 1) / np.float32(1.0 + PAD + EPS))
_C2 = float(np.float32(0.5) * np.float32(RES - 1))
_XMAX = float(np.float32(np.float32(1.0 - EPS) * np.float32(RES - 1)))

# (ia, ib) per plane: u -> x/W axis, v -> y/H axis; pair index = plane*4 + b
_PLANES = [(0, 2), (0, 1), (1, 2)]  # xz, xy, yz

_prog_cache = {}


def _build_program():
    import concourse.bacc as bacc
    import concourse.tile as tile
    import concourse.mybir as mybir

    f32 = mybir.dt.float32
    f32r = mybir.dt.float32r
    bf16 = mybir.dt.bfloat16
    i16 = mybir.dt.int16

    nc = bacc.Bacc(
        "TRN2",
        target_bir_lowering=False,
        debug=False,
        enable_asserts=False,
        num_devices=NCORES,
    )

    p_sw = nc.dram_tensor("p_sw", [128, M * 3], f32, kind="ExternalInput")
    pl_shard = nc.dram_tensor("pl_shard", [12, 128, COLS], f32, kind="ExternalInput")
    w_pl = nc.dram_tensor("w_pl", [128, 1], f32, kind="ExternalInput")
    out_d = nc.dram_tensor("out_sw", [128, M], f32, kind="ExternalOutput")

    with tile.TileContext(nc) as tc:
        with (
            tc.tile_pool(name="const", bufs=1) as constp,
            tc.tile_pool(name="wts", bufs=1) as wts,
            tc.tile_pool(name="tmp", bufs=1) as tmp,
            tc.tile_pool(name="psum", bufs=8, space="PSUM") as psum,
            tc.tile_pool(name="dram", bufs=1, space="DRAM") as dram,
        ):
            # ---------------- phase 1: projection ----------------
            w_tile = constp.tile([128, 1], f32r)
            nc.sync.dma_start(w_tile[:], w_pl.ap().bitcast(f32r))

            shard_bf = dram.tile([12, COLS], bf16)
            with tc.tile_pool(name="ph1", bufs=4) as ph1:
                for j in range(12):
                    for k in range(COLS // 512):
                        chunk = ph1.tile([128, 512], f32r, tag="chunk")
                        src = pl_shard.ap()[j, :, 512 * k : 512 * (k + 1)]
                        nc.sync.dma_start(chunk[:], src.bitcast(f32r))
                        pt = psum.tile([1, 512], f32, tag="pt")
                        nc.tensor.matmul(
                            pt[:], lhsT=w_tile[:], rhs=chunk[:], start=True, stop=True
                        )
                        sb = ph1.tile([1, 512], bf16, tag="projout")
                        nc.scalar.copy(sb[:], pt[:])
                        nc.sync.dma_start(
                            shard_bf[j : j + 1, 512 * k : 512 * (k + 1)], sb[:]
                        )

            # ---------------- phase 2: allgather + pair tables ----------------
            ag_out = dram.tile([NCORES, 12, COLS], bf16)
            nc.gpsimd.collective_compute(
                "AllGather",
                mybir.AluOpType.bypass,
                replica_groups=[list(range(NCORES))],
                ins=[shard_bf.opt()],
                outs=[ag_out.opt()],
            )

            # compact [8,12,COLS] -> [12, 16384] (+2 pad elems)
            t_comp = dram.tile([12, RES * RES + 2], bf16)
            for j in range(12):
                nc.sync.dma_start(t_comp[j : j + 1, 0 : RES * RES], ag_out[:, j, :])

            # pair tables P[s] = (T[s], T[s+1]) built 128-way parallel
            pairs_d = dram.tile([12, RES * RES, 2], bf16)
            with tc.tile_pool(name="pairs", bufs=2) as prp:
                for j in range(12):
                    tsh = prp.tile([128, 129], bf16, tag="tsh")
                    nc.sync.dma_start(
                        tsh[:, 0:128],
                        t_comp[j, 0 : RES * RES].rearrange("(p m) -> p m", p=128),
                    )
                    nc.sync.dma_start(tsh[:, 128:129], t_comp[j, 128 : RES * RES + 2 : 128])
                    pj = prp.tile([128, 128, 2], bf16, tag="pj")
                    nc.vector.tensor_copy(pj[:, :, 0], tsh[:, 0:128])
                    nc.vector.tensor_copy(pj[:, :, 1], tsh[:, 1:129])
                    nc.sync.dma_start(pairs_d[j : j + 1], pj[:])

            # ---------------- phase 3: distribute tables ----------------
            # Two resident table slots per partition; slot 0 holds plane xz then
            # is rewritten with yz, slot 1 holds xy.  Partition 32b+q serves
            # batch b.
            tabgath = tc.tile_pool(name="tables", bufs=1)
            tabp = tabgath.__enter__()
            gath_cm = tc.tile_pool(name="gath", bufs=1)
            gathp = gath_cm.__enter__()
            tabs = tabp.tile([128, 2, RES * RES, 2], bf16)

            def distribute(plane, slot):
                for b in range(B):
                    j = plane * 4 + b
                    for q in range(32):
                        part = 32 * b + q
                        eng = nc.sync if (q % 2 == 0) else nc.scalar
                        eng.dma_start(
                            tabs[part : part + 1, slot], pairs_d[j : j + 1]
                        )

            distribute(0, 0)  # xz -> slot 0
            distribute(1, 1)  # xy -> slot 1

            # ---------------- phase 4: sampling ----------------
            p_sb = constp.tile([128, M, 3], f32)
            nc.sync.dma_start(p_sb[:], p_sw.ap())

            acc = constp.tile([128, M], f32)

            for pli, (ia, ib) in enumerate(_PLANES):
                slot = [0, 1, 0][pli]
                if pli == 2:
                    distribute(2, 0)  # yz overwrites slot 0 (after xz reads)

                # weights / indices, all M slots at once
                xt = tmp.tile([128, M], f32, tag="sc0")
                nc.vector.tensor_scalar(
                    xt[:], p_sb[:, :, ia], _C1, _C2,
                    mybir.AluOpType.mult, mybir.AluOpType.add,
                )
                nc.vector.tensor_scalar(
                    xt[:], xt[:], 0.0, _XMAX,
                    mybir.AluOpType.max, mybir.AluOpType.min,
                )
                # floor(x) for x>=0, robust to either trunc or round-nearest casts:
                # xi = int(x); x0 = xi - (xi > x); wx = x - x0
                xi = tmp.tile([128, M], mybir.dt.int32, tag="xi")
                nc.vector.tensor_copy(xi[:], xt[:])
                xf = tmp.tile([128, M], f32, tag="xf")
                nc.vector.tensor_copy(xf[:], xi[:])
                mk = tmp.tile([128, M], f32, tag="mk")
                nc.vector.tensor_tensor(mk[:], xf[:], xt[:], mybir.AluOpType.is_gt)
                x0 = tmp.tile([128, M], f32, tag="sc1")
                nc.vector.tensor_tensor(x0[:], xf[:], mk[:], mybir.AluOpType.subtract)
                wx = wts.tile([128, M], f32, tag="wx")
                nc.vector.tensor_tensor(wx[:], xt[:], x0[:], mybir.AluOpType.subtract)

                yt = tmp.tile([128, M], f32, tag="sc2")
                nc.scalar.activation(
                    yt[:], p_sb[:, :, ib], mybir.ActivationFunctionType.Copy,
                    bias=_C2, scale=_C1,
                )
                nc.vector.tensor_scalar(
                    yt[:], yt[:], 0.0, _XMAX,
                    mybir.AluOpType.max, mybir.AluOpType.min,
                )
                nc.vector.tensor_copy(xi[:], yt[:])
                nc.vector.tensor_copy(xf[:], xi[:])
                nc.vector.tensor_tensor(mk[:], xf[:], yt[:], mybir.AluOpType.is_gt)
                y0 = tmp.tile([128, M], f32, tag="sc3")
                nc.vector.tensor_tensor(y0[:], xf[:], mk[:], mybir.AluOpType.subtract)
                wy = wts.tile([128, M], f32, tag="wy")
                nc.vector.tensor_tensor(wy[:], yt[:], y0[:], mybir.AluOpType.subtract)
                # s = y0*128 + x0
                st = tmp.tile([128, M], f32, tag="sc4")
                nc.vector.tensor_scalar(
                    st[:], y0[:], float(RES), None, mybir.AluOpType.mult
                )
                nc.vector.tensor_tensor(st[:], st[:], x0[:], mybir.AluOpType.add)

                idx0 = wts.tile([128, M], i16, tag="idx0")
                nc.vector.tensor_copy(idx0[:], st[:])
                nc.vector.tensor_scalar(
                    st[:], st[:], float(RES), None, mybir.AluOpType.add
                )
                idx1 = wts.tile([128, M], i16, tag="idx1")
                nc.vector.tensor_copy(idx1[:], st[:])

                # gathers: one instruction per row
                g0 = gathp.tile([128, J, 2], bf16, tag="g0")
                nc.gpsimd.ap_gather(
                    g0[:], tabs[:, slot], idx0[:],
                    channels=128, num_elems=RES * RES, d=2, num_idxs=J,
                )
                g1 = gathp.tile([128, J, 2], bf16, tag="g1")
                nc.gpsimd.ap_gather(
                    g1[:], tabs[:, slot], idx1[:],
                    channels=128, num_elems=RES * RES, d=2, num_idxs=J,
                )

                # de-diagonalize: a[p, m, :] = g[p, 16m + p%16, :]
                a0 = tmp.tile([128, M, 2], bf16, tag="a0")
                a1 = tmp.tile([128, M, 2], bf16, tag="a1")
                for r in range(16):
                    nc.sync.dma_start(a0[r::16], g0[r::16, r::16, :])
                    nc.scalar.dma_start(a1[r::16], g1[r::16, r::16, :])

                # combine (f32): l0/l1 row lerps, then y lerp
                d0 = tmp.tile([128, M], f32, tag="c0")
                nc.vector.tensor_tensor(d0[:], a0[:, :, 1], a0[:, :, 0], mybir.AluOpType.subtract)
                nc.vector.tensor_tensor(d0[:], d0[:], wx[:], mybir.AluOpType.mult)
                nc.vector.tensor_tensor(d0[:], d0[:], a0[:, :, 0], mybir.AluOpType.add)

                d1 = tmp.tile([128, M], f32, tag="c1")
                nc.vector.tensor_tensor(d1[:], a1[:, :, 1], a1[:, :, 0], mybir.AluOpType.subtract)
                nc.vector.tensor_tensor(d1[:], d1[:], wx[:], mybir.AluOpType.mult)
                nc.vector.tensor_tensor(d1[:], d1[:], a1[:, :, 0], mybir.AluOpType.add)

                nc.vector.tensor_tensor(d1[:], d1[:], d0[:], mybir.AluOpType.subtract)
                nc.vector.tensor_tensor(d1[:], d1[:], wy[:], mybir.AluOpType.mult)
                nc.vector.tensor_tensor(d1[:], d1[:], d0[:], mybir.AluOpType.add)
                if pli == 0:
                    nc.vector.tensor_copy(acc[:], d1[:])
                else:
                    nc.vector.tensor_tensor(acc[:], acc[:], d1[:], mybir.AluOpType.add)

            nc.sync.dma_start(out_d.ap(), acc[:])
            gath_cm.__exit__(None, None, None)
            tabgath.__exit__(None, None, None)

    nc.compile()
    return nc


def _get_program():
    if "nc" not in _prog_cache:
        _prog_cache["nc"] = _build_program()
    return _prog_cache["nc"]


def kernel(p, c_xz, c_xy, c_yz, fc_w, fc_b, trace=False):
    from concourse import bass_utils

    nc = _get_program()

    p = np.asarray(p, dtype=np.float32)
    fc_w = np.asarray(fc_w, dtype=np.float32)
    fc_b = np.asarray(fc_b, dtype=np.float32)

    planes12 = np.empty((12, 128, RES * RES), dtype=np.float32)
    for pli, c in enumerate([c_xz, c_xy, c_yz]):
        c = np.asarray(c, dtype=np.float32)
        planes12[pli * 4 : pli * 4 + 4] = c.reshape(B, C, RES * RES)

    w_pl_np = np.ascontiguousarray(fc_w[:128].reshape(128, 1))

    in_maps = []
    for r in range(NCORES):
        p_r = p[:, r * NP : (r + 1) * NP, :]
        p_pad = np.zeros((B, NPAD, 3), dtype=np.float32)
        p_pad[:, :NP] = p_r
        p_swz = np.ascontiguousarray(
            p_pad.reshape(B, M, 32, 3).transpose(0, 2, 1, 3).reshape(128, M * 3)
        )
        in_maps.append(
            {
                "p_sw": p_swz,
                "pl_shard": np.ascontiguousarray(
                    planes12[:, :, r * COLS : (r + 1) * COLS]
                ),
                "w_pl": w_pl_np,
            }
        )

    res = bass_utils.run_bass_kernel_spmd(
        nc, in_maps, core_ids=list(range(NCORES)), trace=trace
    )
    if trace:
        print("exec_time_ns:", res.exec_time_ns)
        kernel.last_results = res

    out = np.empty((B, N), dtype=np.float32)
    for r in range(NCORES):
        o = res.results[r]["out_sw"]
        o = o.reshape(B, 32, M).transpose(0, 2, 1).reshape(B, NPAD)
        out[:, r * NP : (r + 1) * NP] = o[:, :NP]

    out += p @ fc_w[128:131, 0] + fc_b[0]
    return out
